# revision 3
# baseline (speedup 1.0000x reference)
"""MHSA block (patch-embed conv + relative-pos attention + MLP) on 8 NeuronCores.

Sharding: pure data-parallel over batch (64 images -> 8 per core), weights
replicated. Host does only layout prep (transposes/casts/rel-pos gather);
all model compute runs on-device via Bass/Tile.
"""
import numpy as np
import ml_dtypes
import concourse.bass as bass
import concourse.bacc as bacc
import concourse.tile as tile
from concourse import mybir
from concourse import bass_utils
from concourse.masks import make_identity

BF = ml_dtypes.bfloat16
B, CIN, D, HEADS, HD = 64, 384, 768, 12, 64
GS, ET, N = 16, 1, 257
BL = B // 8              # images per core
NT = BL * N              # 2056 packed tokens per core
MLP = 4 * D
CHUNKS = [(i * 128, min(128, NT - i * 128)) for i in range(17)]
COLT = [(c, min(512, NT - c)) for c in range(0, NT, 512)]
QCH = [(0, 86), (86, 86), (172, 85)]
MCH = [(0, 128), (128, 128), (256, 1)]

_CACHE = {}
_LAST_MAPS = None


def _rel_bias(rpb_table):
    coords = np.stack(np.meshgrid(np.arange(GS), np.arange(GS), indexing='ij'))
    cf = coords.reshape(2, -1)
    rel = (cf[:, :, None] - cf[:, None, :]).transpose(1, 2, 0)
    rel[:, :, 0] += GS - 1
    rel[:, :, 1] += GS - 1
    rel[:, :, 0] *= 2 * GS - 1
    idx = rel.sum(-1)
    out = np.zeros((N, N), dtype=np.int32)
    out[ET:, ET:] = idx
    bias = rpb_table[out]                    # [N, N, HEADS]
    return bias.transpose(2, 0, 1).astype(np.float32)   # [HEADS, N, N]


def _ln_pair(tc, nc, pools, xt, ts):
    """mean/rstd of xt[:ts, :768] -> (mean, rstd) [ts,1] f32 tiles."""
    st = pools.tile([128, 3, nc.vector.BN_STATS_DIM], mybir.dt.float32, tag="lnst")
    xg = xt.rearrange("p (n f) -> p n f", f=256)
    for i in range(3):
        nc.vector.bn_stats(out=st[:ts, i], in_=xg[:ts, i])
    mv = pools.tile([128, nc.vector.BN_AGGR_DIM], mybir.dt.float32, tag="lnmv")
    nc.vector.bn_aggr(out=mv[:ts], in_=st[:ts])
    eps = pools.tile([128, 1], mybir.dt.float32, tag="lneps")
    nc.vector.memset(eps, 1e-5)
    rs = pools.tile([128, 1], mybir.dt.float32, tag="lnrs")
    nc.scalar.activation(out=rs[:ts], in_=mv[:ts, 1:2],
                         func=mybir.ActivationFunctionType.Sqrt, bias=eps[:ts])
    nc.vector.reciprocal(out=rs[:ts], in_=rs[:ts])
    return mv, rs


def build():
    nc = bacc.Bacc("TRN2", target_bir_lowering=False, debug=False)
    f32, bf16 = mybir.dt.float32, mybir.dt.bfloat16
    di = lambda n, s, d: nc.dram_tensor(n, s, d, kind="ExternalInput").ap()
    x_in = di("x_in", [BL, 3, 128, 32, 32], bf16)
    convw = di("convw", [27, 128, 768], bf16)
    convb_bc = di("convb_bc", [128, 768], f32)
    peg_bc = di("peg_bc", [128, 768], f32)
    geo2 = di("geo2", [2, 128, 768], f32)
    y0row = di("y0row", [1, 768], f32)
    qkvw = di("qkvw", [6, 128, 2304], bf16)
    qkvb_t = di("qkvb_t", [128, 18], f32)
    projw = di("projw", [6, 128, 768], bf16)
    projb_bc = di("projb_bc", [128, 768], f32)
    fc1w = di("fc1w", [6, 128, MLP], bf16)
    fc1b_t = di("fc1b_t", [128, 24], f32)
    fc2w = di("fc2w", [24, 128, 768], bf16)
    fc2b_bc = di("fc2b_bc", [128, 768], f32)
    bias4 = di("bias4", [12, 86, 3, 257], f32)
    out_d = nc.dram_tensor("out_d", [NT, 768], f32, kind="ExternalOutput").ap()

    with tile.TileContext(nc) as tc:
        with tc.tile_pool(name="dram", bufs=1, space="DRAM") as dpool:
            y_d = dpool.tile([NT, 768], f32)
            hT_d = dpool.tile([6, 128, NT], bf16)
            qkT_d = dpool.tile([18, 128, NT], bf16)
            oT_d = dpool.tile([6, 128, NT], bf16)
            y2_d = dpool.tile([NT, 768], f32)
            h2T_d = dpool.tile([6, 128, NT], bf16)
            h3T_d = dpool.tile([24, 128, NT], bf16)

            # ---------------- Phase 1: conv + peLN + geo -> y_d ----------------
            with tc.tile_pool(name="cw", bufs=1) as cw, \
                 tc.tile_pool(name="cx", bufs=2) as cx, \
                 tc.tile_pool(name="cps", bufs=4, space="PSUM") as cps, \
                 tc.tile_pool(name="cy", bufs=3) as cy:
                wsb = cw.tile([128, 27, 768], bf16)
                for i in range(27):
                    nc.sync.dma_start(out=wsb[:, i], in_=convw[i])
                cbc = cw.tile([128, 768], f32)
                nc.sync.dma_start(out=cbc, in_=convb_bc)
                pgc = cw.tile([128, 768], f32)
                nc.sync.dma_start(out=pgc, in_=peg_bc)
                gsb = cw.tile([128, 2, 768], f32)
                for t in range(2):
                    nc.sync.dma_start(out=gsb[:, t], in_=geo2[t])
                y0sb = cw.tile([1, 768], f32)
                nc.sync.dma_start(out=y0sb, in_=y0row)
                for b in range(BL):
                    nc.sync.dma_start(out=y_d[b * N:b * N + 1, :], in_=y0sb)
                for b in range(BL):
                    xp = cx.tile([128, 3, 1089], bf16, tag="xpad")
                    nc.vector.memset(xp, 0.0)
                    for c in range(3):
                        dst = bass.AP(tensor=xp.tensor, offset=xp.offset + c * 1089 + 34,
                                      ap=[xp.ap[0], [33, 32], [1, 32]])
                        nc.sync.dma_start(out=dst, in_=x_in[b, c])
                    for t in range(2):
                        col = cx.tile([128, 27, 128], bf16, tag="col")
                        for kh in range(3):
                            for kw in range(3):
                                for c in range(3):
                                    idx = (kh * 3 + kw) * 3 + c
                                    src = bass.AP(
                                        tensor=xp.tensor,
                                        offset=xp.offset + c * 1089 + (16 * t + kh) * 33 + kw,
                                        ap=[xp.ap[0], [66, 8], [2, 16]])
                                    nc.vector.tensor_copy(col[:, idx], src)
                        yt = cy.tile([128, 768], f32, tag="yt")
                        for nh in range(2):
                            ps = cps.tile([128, 384], f32, tag="cpsum")
                            for i in range(27):
                                nc.tensor.matmul(ps, col[:, i], wsb[:, i, nh * 384:(nh + 1) * 384],
                                                 start=(i == 0), stop=(i == 26))
                            nc.vector.tensor_add(yt[:, nh * 384:(nh + 1) * 384], ps, cbc[:, nh * 384:(nh + 1) * 384])
                        mv, rs = _ln_pair(tc, nc, cy, yt, 128)
                        nc.vector.tensor_scalar(out=yt, in0=yt, scalar1=mv[:, 0:1], scalar2=rs,
                                                op0=mybir.AluOpType.subtract, op1=mybir.AluOpType.mult)
                        nc.vector.tensor_mul(yt, yt, pgc)
                        nc.vector.tensor_add(yt, yt, gsb[:, t])
                        r0 = b * N + 1 + t * 128
                        nc.sync.dma_start(out=y_d[r0:r0 + 128, :], in_=yt)

            # ---------------- Phase 2: LN1 + transpose -> hT_d ----------------
            with tc.tile_pool(name="l1", bufs=3) as l1, \
                 tc.tile_pool(name="l1c", bufs=1) as l1c, \
                 tc.tile_pool(name="l1p", bufs=4, space="PSUM") as l1p:
                idb = l1c.tile([128, 128], bf16)
                make_identity(nc, idb)
                for (t0, ts) in CHUNKS:
                    yt = l1.tile([128, 768], f32, tag="l1y")
                    nc.sync.dma_start(out=yt[:ts], in_=y_d[t0:t0 + ts, :])
                    mv, rs = _ln_pair(tc, nc, l1, yt, ts)
                    hb = l1.tile([128, 768], bf16, tag="l1h")
                    nc.vector.tensor_scalar(out=hb[:ts], in0=yt[:ts], scalar1=mv[:ts, 0:1], scalar2=rs[:ts],
                                            op0=mybir.AluOpType.subtract, op1=mybir.AluOpType.mult)
                    for k in range(6):
                        tp = l1p.tile([128, 128], bf16, tag="l1t")
                        nc.tensor.transpose(tp[:, :ts], hb[:ts, k * 128:(k + 1) * 128], idb[:ts, :ts])
                        st = l1.tile([128, 128], bf16, tag="l1s")
                        nc.vector.tensor_copy(st[:, :ts], tp[:, :ts])
                        nc.sync.dma_start(out=hT_d[k, :, t0:t0 + ts], in_=st[:, :ts])

            # ---------------- Phase 3: QKV -> qkT_d ----------------
            with tc.tile_pool(name="qw", bufs=1) as qw, \
                 tc.tile_pool(name="qa", bufs=3) as qa, \
                 tc.tile_pool(name="qp", bufs=8, space="PSUM") as qp:
                wq = qw.tile([128, 6, 2304], bf16)
                for k in range(6):
                    nc.sync.dma_start(out=wq[:, k], in_=qkvw[k])
                qb = qw.tile([128, 18], f32)
                nc.sync.dma_start(out=qb, in_=qkvb_t)
                for (c0, cs) in COLT:
                    ht = qa.tile([128, 6, 512], bf16, tag="qh")
                    for k in range(6):
                        nc.sync.dma_start(out=ht[:, k, :cs], in_=hT_d[k, :, c0:c0 + cs])
                    for dch in range(18):
                        ps = qp.tile([128, 512], f32, tag="qps")
                        for k in range(6):
                            nc.tensor.matmul(ps[:, :cs], wq[:, k, dch * 128:(dch + 1) * 128],
                                             ht[:, k, :cs], start=(k == 0), stop=(k == 5))
                        ev = qa.tile([128, 512], bf16, tag="qev")
                        nc.vector.tensor_scalar_add(out=ev[:, :cs], in0=ps[:, :cs], scalar1=qb[:, dch:dch + 1])
                        nc.sync.dma_start(out=qkT_d[dch, :, c0:c0 + cs], in_=ev[:, :cs])

            # ---------------- Phase 4: attention -> oT_d ----------------
            with tc.tile_pool(name="ac", bufs=1) as ac, \
                 tc.tile_pool(name="ab", bufs=2) as ab, \
                 tc.tile_pool(name="aw", bufs=3) as aw, \
                 tc.tile_pool(name="ap", bufs=2, space="PSUM") as app:
                idb = ac.tile([128, 128], bf16)
                make_identity(nc, idb)
                for h in range(12):
                    bsb = ab.tile([86, 3, 257], f32, tag="bias")
                    nc.sync.dma_start(out=bsb, in_=bias4[h])
                    for b in range(BL):
                        qt = aw.tile([64, 257], bf16, tag="qt")
                        kt = aw.tile([64, 257], bf16, tag="kt")
                        vt = aw.tile([64, 257], bf16, tag="vt")
                        po = (h % 2) * 64
                        nc.sync.dma_start(out=qt, in_=qkT_d[h // 2, po:po + 64, b * N:b * N + N])
                        nc.sync.dma_start(out=kt, in_=qkT_d[6 + h // 2, po:po + 64, b * N:b * N + N])
                        nc.sync.dma_start(out=vt, in_=qkT_d[12 + h // 2, po:po + 64, b * N:b * N + N])
                        vsb = aw.tile([128, 3, 64], bf16, tag="vsb")
                        for mi, (mo, ms) in enumerate(MCH):
                            tp = app.tile([128, 64], bf16, tag="vtp")
                            nc.tensor.transpose(tp[:ms, :], vt[:, mo:mo + ms], idb[:64, :64])
                            nc.vector.tensor_copy(vsb[:ms, mi], tp[:ms, :])
                        for (q0, qs) in QCH:
                            sp = app.tile([128, 257], f32, tag="sps")
                            nc.tensor.matmul(sp[:qs], qt[:, q0:q0 + qs], kt, start=True, stop=True)
                            sc = aw.tile([86, 257], f32, tag="sc")
                            nc.vector.tensor_add(sc[:qs], sp[:qs], bsb[:qs, QCH.index((q0, qs))])
                            pr = aw.tile([86, 257], bf16, tag="pr")
                            rsum = aw.tile([86, 1], f32, tag="rsum")
                            nc.scalar.activation(pr[:qs], sc[:qs], mybir.ActivationFunctionType.Exp,
                                                 accum_out=rsum[:qs])
                            nc.vector.reciprocal(rsum[:qs], rsum[:qs])
                            nc.vector.tensor_scalar_mul(pr[:qs], pr[:qs], rsum[:qs])
                            pT = aw.tile([128, 3, 86], bf16, tag="pT")
                            for mi, (mo, ms) in enumerate(MCH):
                                tp2 = app.tile([128, 86], bf16, tag="ptp")
                                nc.tensor.transpose(tp2[:ms, :qs], pr[:qs, mo:mo + ms], idb[:qs, :qs])
                                nc.vector.tensor_copy(pT[:ms, mi, :qs], tp2[:ms, :qs])
                            op = app.tile([64, 86], f32, tag="ops")
                            for mi, (mo, ms) in enumerate(MCH):
                                nc.tensor.matmul(op[:, :qs], vsb[:ms, mi], pT[:ms, mi, :qs],
                                                 start=(mi == 0), stop=(mi == 2))
                            oe = aw.tile([64, 86], bf16, tag="oe")
                            nc.vector.tensor_copy(oe[:, :qs], op[:, :qs])
                            nc.sync.dma_start(out=oT_d[h // 2, po:po + 64, b * N + q0:b * N + q0 + qs],
                                              in_=oe[:, :qs])

            # ---------------- Phase 5: proj + residual + LN2 + T -> y2_d, h2T_d ----------------
            with tc.tile_pool(name="pw", bufs=1) as pw, \
                 tc.tile_pool(name="pa", bufs=3) as pa, \
                 tc.tile_pool(name="pp", bufs=4, space="PSUM") as pp:
                wp = pw.tile([128, 6, 768], bf16)
                for k in range(6):
                    nc.sync.dma_start(out=wp[:, k], in_=projw[k])
                pbc = pw.tile([128, 768], f32)
                nc.sync.dma_start(out=pbc, in_=projb_bc)
                idb2 = pw.tile([128, 128], bf16)
                make_identity(nc, idb2)
                for (t0, ts) in CHUNKS:
                    ot = pa.tile([128, 6, 128], bf16, tag="pot")
                    for k in range(6):
                        nc.sync.dma_start(out=ot[:, k, :ts], in_=oT_d[k, :, t0:t0 + ts])
                    yt = pa.tile([128, 768], f32, tag="py")
                    nc.sync.dma_start(out=yt[:ts], in_=y_d[t0:t0 + ts, :])
                    y2 = pa.tile([128, 768], f32, tag="py2")
                    for nh in range(2):
                        ps = pp.tile([128, 384], f32, tag="pps")
                        for k in range(6):
                            nc.tensor.matmul(ps[:ts], ot[:, k, :ts], wp[:, k, nh * 384:(nh + 1) * 384],
                                             start=(k == 0), stop=(k == 5))
                        nc.vector.tensor_add(y2[:ts, nh * 384:(nh + 1) * 384], ps[:ts],
                                             yt[:ts, nh * 384:(nh + 1) * 384])
                    nc.vector.tensor_add(y2[:ts], y2[:ts], pbc[:ts])
                    nc.sync.dma_start(out=y2_d[t0:t0 + ts, :], in_=y2[:ts])
                    mv, rs = _ln_pair(tc, nc, pa, y2, ts)
                    hb = pa.tile([128, 768], bf16, tag="ph2")
                    nc.vector.tensor_scalar(out=hb[:ts], in0=y2[:ts], scalar1=mv[:ts, 0:1], scalar2=rs[:ts],
                                            op0=mybir.AluOpType.subtract, op1=mybir.AluOpType.mult)
                    for k in range(6):
                        tp = pp.tile([128, 128], bf16, tag="ptr")
                        nc.tensor.transpose(tp[:, :ts], hb[:ts, k * 128:(k + 1) * 128], idb2[:ts, :ts])
                        st = pa.tile([128, 128], bf16, tag="pst")
                        nc.vector.tensor_copy(st[:, :ts], tp[:, :ts])
                        nc.sync.dma_start(out=h2T_d[k, :, t0:t0 + ts], in_=st[:, :ts])

            # ---------------- Phase 6: FC1 + gelu -> h3T_d ----------------
            with tc.tile_pool(name="f1w", bufs=1) as f1w, \
                 tc.tile_pool(name="f1a", bufs=3) as f1a, \
                 tc.tile_pool(name="f1p", bufs=8, space="PSUM") as f1p:
                w1 = f1w.tile([128, 6, MLP], bf16)
                for k in range(6):
                    nc.sync.dma_start(out=w1[:, k], in_=fc1w[k])
                b1 = f1w.tile([128, 24], f32)
                nc.sync.dma_start(out=b1, in_=fc1b_t)
                for (c0, cs) in COLT:
                    ht = f1a.tile([128, 6, 512], bf16, tag="f1h")
                    for k in range(6):
                        nc.sync.dma_start(out=ht[:, k, :cs], in_=h2T_d[k, :, c0:c0 + cs])
                    for dch in range(24):
                        ps = f1p.tile([128, 512], f32, tag="f1ps")
                        for k in range(6):
                            nc.tensor.matmul(ps[:, :cs], w1[:, k, dch * 128:(dch + 1) * 128],
                                             ht[:, k, :cs], start=(k == 0), stop=(k == 5))
                        ev = f1a.tile([128, 512], bf16, tag="f1ev")
                        nc.scalar.activation(ev[:, :cs], ps[:, :cs], mybir.ActivationFunctionType.Gelu,
                                             bias=b1[:, dch:dch + 1])
                        nc.sync.dma_start(out=h3T_d[dch, :, c0:c0 + cs], in_=ev[:, :cs])

            # ---------------- Phase 7: FC2 + residual -> out ----------------
            with tc.tile_pool(name="f2w", bufs=1) as f2w, \
                 tc.tile_pool(name="f2a", bufs=3) as f2a, \
                 tc.tile_pool(name="f2p", bufs=8, space="PSUM") as f2p:
                w2 = f2w.tile([128, 24, 768], bf16)
                for k in range(24):
                    nc.sync.dma_start(out=w2[:, k], in_=fc2w[k])
                b2c = f2w.tile([128, 768], f32)
                nc.sync.dma_start(out=b2c, in_=fc2b_bc)
                for (t0, ts) in CHUNKS:
                    h3 = f2a.tile([128, 24, 128], bf16, tag="f2h")
                    for k in range(24):
                        nc.sync.dma_start(out=h3[:, k, :ts], in_=h3T_d[k, :, t0:t0 + ts])
                    y2 = f2a.tile([128, 768], f32, tag="f2y")
                    nc.sync.dma_start(out=y2[:ts], in_=y2_d[t0:t0 + ts, :])
                    ot = f2a.tile([128, 768], f32, tag="f2o")
                    for nh in range(2):
                        ps = f2p.tile([128, 384], f32, tag="f2ps")
                        for k in range(24):
                            nc.tensor.matmul(ps[:ts], h3[:, k, :ts], w2[:, k, nh * 384:(nh + 1) * 384],
                                             start=(k == 0), stop=(k == 23))
                        nc.vector.tensor_add(ot[:ts, nh * 384:(nh + 1) * 384], ps[:ts],
                                             y2[:ts, nh * 384:(nh + 1) * 384])
                    nc.vector.tensor_add(ot[:ts], ot[:ts], b2c[:ts])
                    nc.sync.dma_start(out=out_d[t0:t0 + ts, :], in_=ot[:ts])

    nc.compile()
    return nc


def kernel(x, H, W, geo_bias, extra_token, conv_w, conv_b, pe_g, pe_b,
           n1_g, n1_b, qkv_w, rpb_table, proj_w, proj_b, n2_g, n2_b,
           fc1_w, fc1_b, fc2_w, fc2_b):
    x = np.asarray(x, np.float32)
    f = lambda a: np.asarray(a, np.float32)
    geo_bias, extra_token = f(geo_bias), f(extra_token)
    conv_w, conv_b, pe_g, pe_b = f(conv_w), f(conv_b), f(pe_g), f(pe_b)
    n1_g, n1_b, qkv_w, rpb_table = f(n1_g), f(n1_b), f(qkv_w), f(rpb_table)
    proj_w, proj_b, n2_g, n2_b = f(proj_w), f(proj_b), f(n2_g), f(n2_b)
    fc1_w, fc1_b, fc2_w, fc2_b = f(fc1_w), f(fc1_b), f(fc2_w), f(fc2_b)

    if "nc" not in _CACHE:
        _CACHE["nc"] = build()
    nc = _CACHE["nc"]

    # host-side weight prep (layout only; LN scale folds are exact for g=1,b=0)
    cw = conv_w.transpose(2, 3, 1, 0).reshape(3, 3, 3, 128, 768).reshape(27, 128, 768)
    qkv_wf = qkv_w * n1_g[None, :]
    qkv_wf[:D] *= HD ** -0.5
    qkv_b = qkv_w @ n1_b
    qkv_b[:D] *= HD ** -0.5
    fc1_wf = fc1_w * n2_g[None, :]
    fc1_bf = fc1_b + fc1_w @ n2_b
    bias_full = _rel_bias(rpb_table)
    b4 = np.zeros((12, 86, 3, 257), np.float32)
    for qc, (q0, qs) in enumerate(QCH):
        b4[:, :qs, qc, :] = bias_full[:, q0:q0 + qs, :]

    common = {
        "convw": cw.astype(BF),
        "convb_bc": np.tile(conv_b[None, :], (128, 1)).astype(np.float32),
        "peg_bc": np.tile(pe_g[None, :], (128, 1)).astype(np.float32),
        "geo2": (geo_bias[0, 1:, :] + pe_b[None, :]).reshape(2, 128, 768).astype(np.float32),
        "y0row": (extra_token[0] + geo_bias[0, :1, :]).astype(np.float32),
        "qkvw": qkv_wf.T.reshape(6, 128, 2304).astype(BF),
        "qkvb_t": np.ascontiguousarray(qkv_b.reshape(18, 128).T).astype(np.float32),
        "projw": proj_w.T.reshape(6, 128, 768).astype(BF),
        "projb_bc": np.tile(proj_b[None, :], (128, 1)).astype(np.float32),
        "fc1w": fc1_wf.T.reshape(6, 128, MLP).astype(BF),
        "fc1b_t": np.ascontiguousarray(fc1_bf.reshape(24, 128).T).astype(np.float32),
        "fc2w": fc2_w.T.reshape(24, 128, 768).astype(BF),
        "fc2b_bc": np.tile(fc2_b[None, :], (128, 1)).astype(np.float32),
        "bias4": b4,
    }
    in_maps = []
    for c in range(8):
        xs = x[c * BL:(c + 1) * BL].reshape(BL, 3, 128, 32, 32).astype(BF)
        in_maps.append({"x_in": xs, **common})

    global _LAST_MAPS
    _LAST_MAPS = in_maps
    res = bass_utils.run_bass_kernel_spmd(nc, in_maps, core_ids=list(range(8)))
    out = np.concatenate([r["out_d"].reshape(BL, N, D) for r in res.results], axis=0)
    return out.astype(np.float32)



# revision 4
# speedup vs baseline: 1.0027x; 1.0027x over previous
"""MHSA block (patch-embed conv + relative-pos attention + MLP) on 8 NeuronCores.

Data-parallel over batch (8 images/core). Fully SBUF-resident pipeline:
conv -> peLN+geo -> LN1 -> QKV -> attention -> proj+residual -> LN2 -> MLP.

Internal token order per core ("perm-1"): main tokens j = b*256 + (n-1)
for n in [1,257) (2048 tokens, 128-aligned chunks), extra tokens at rows
[2048, 2056). Host un-permutes the output.
"""
import numpy as np
import ml_dtypes
import concourse.bass as bass
import concourse.bacc as bacc
import concourse.tile as tile
from concourse import mybir
from concourse import bass_utils
from concourse.masks import make_identity

BF = ml_dtypes.bfloat16
B, CIN, D, HEADS, HD = 64, 384, 768, 12, 64
GS, ET, N = 16, 1, 257
BL = 8
NMAIN = BL * 256          # 2048
NT = BL * N               # 2056
MLP = 4 * D
F32 = mybir.dt.float32
BF16 = mybir.dt.bfloat16
AF = mybir.ActivationFunctionType
ALU = mybir.AluOpType

_CACHE = {}
_LAST_MAPS = None


def _rel_bias(rpb_table):
    coords = np.stack(np.meshgrid(np.arange(GS), np.arange(GS), indexing='ij'))
    cf = coords.reshape(2, -1)
    rel = (cf[:, :, None] - cf[:, None, :]).transpose(1, 2, 0)
    rel[:, :, 0] += GS - 1
    rel[:, :, 1] += GS - 1
    rel[:, :, 0] *= 2 * GS - 1
    idx = rel.sum(-1)
    out = np.zeros((N, N), dtype=np.int32)
    out[ET:, ET:] = idx
    bias = rpb_table[out]                    # [N, N, HEADS] (q, k, h)
    return bias.transpose(2, 0, 1).astype(np.float32)   # [HEADS, q, k]


def _ln_stats(nc, pool, g0, g1, ts, eps):
    """LN stats over 768 free elems given two [*,384] group APs -> rstd, mean."""
    st = pool.tile([128, 2, nc.vector.BN_STATS_DIM], F32, tag="lnst")
    nc.vector.bn_stats(out=st[:ts, 0], in_=g0)
    nc.vector.bn_stats(out=st[:ts, 1], in_=g1)
    mv = pool.tile([128, nc.vector.BN_AGGR_DIM], F32, tag="lnmv")
    nc.vector.bn_aggr(out=mv[:ts], in_=st[:ts])
    rs = pool.tile([128, 1], F32, tag="lnrs")
    nc.scalar.activation(out=rs[:ts], in_=mv[:ts, 1:2], func=AF.Sqrt, bias=eps[:ts])
    nc.vector.reciprocal(out=rs[:ts], in_=rs[:ts])
    return mv, rs


def build():
    nc = bacc.Bacc("TRN2", target_bir_lowering=False, debug=False)
    di = lambda n, s, d: nc.dram_tensor(n, s, d, kind="ExternalInput").ap()
    x_col = di("x_col", [BL, 27, 128, 256], BF16)
    convw = di("convw", [27, 128, 768], BF16)
    convb = di("convb", [1, 768], BF16)
    peg_bc = di("peg_bc", [128, 768], F32)
    geo2 = di("geo2", [2, 128, 768], F32)
    y0row = di("y0row", [1, 768], F32)
    qkvw = di("qkvw", [6, 128, 2304], BF16)
    qkvb_t = di("qkvb_t", [128, 18], F32)
    vb_d = di("vb", [1, 768], BF16)
    e2_d = di("e2", [2, 128], BF16)
    bT_d = di("bT_d", [12, 257, 257], BF16)     # [h, k', q'] permuted bias
    projw = di("projw", [6, 128, 768], BF16)
    projb = di("projb", [1, 768], BF16)
    fc1w = di("fc1w", [6, 128, MLP], BF16)
    fc1b_t = di("fc1b_t", [128, 24], F32)
    fc2w = di("fc2w", [24, 128, 768], BF16)
    fc2b = di("fc2b", [1, 768], BF16)
    out_d = nc.dram_tensor("out_d", [NT, 768], F32, kind="ExternalOutput").ap()

    CH = [(c * 128, 128) for c in range(16)] + [(2048, 8)]   # token chunks
    CC = [(0, 512), (512, 512), (1024, 512), (1536, 512), (2048, 8)]

    with tile.TileContext(nc) as tc:
        pc_cm = tc.tile_pool(name="const", bufs=1)
        pc = pc_cm.__enter__()
        idb = pc.tile([128, 128], BF16)
        make_identity(nc, idb)
        ones1 = pc.tile([1, 128], BF16)
        nc.vector.memset(ones1, 1.0)
        ones65 = pc.tile([65, 64], BF16)
        nc.vector.memset(ones65, 1.0)
        eps = pc.tile([128, 1], F32)
        nc.vector.memset(eps, 1e-5)

        pr_cm = tc.tile_pool(name="resid", bufs=1)
        pr = pr_cm.__enter__()
        ym = pr.tile([128, 16, 768], F32)
        yx = pr.tile([8, 768], F32)

        pht_cm = tc.tile_pool(name="htp", bufs=1)
        pht = pht_cm.__enter__()
        hT = pht.tile([128, 6, 2056], BF16)

        # ---------------- P1: conv + peLN + geo -> ym / yx ----------------
        cw_cm = tc.tile_pool(name="cw", bufs=1)
        cx_cm = tc.tile_pool(name="cx", bufs=2)
        cps_cm = tc.tile_pool(name="cps", bufs=2, space="PSUM")
        csc_cm = tc.tile_pool(name="csc", bufs=3)
        cw, cx, cps, csc = (cm.__enter__() for cm in (cw_cm, cx_cm, cps_cm, csc_cm))
        wsb = cw.tile([128, 27, 768], BF16)
        for i in range(27):
            nc.sync.dma_start(out=wsb[:, i], in_=convw[i])
        cbsb = cw.tile([1, 768], BF16)
        nc.sync.dma_start(out=cbsb, in_=convb)
        pgc = cw.tile([128, 768], F32)
        nc.sync.dma_start(out=pgc, in_=peg_bc)
        gsb = cw.tile([128, 2, 768], F32)
        for t in range(2):
            nc.sync.dma_start(out=gsb[:, t], in_=geo2[t])
        for b in range(BL):
            nc.sync.dma_start(out=yx[b:b + 1, :], in_=y0row)
        for b in range(BL):
            xp = cx.tile([128, 27, 256], BF16, tag="xp")
            for i in range(27):
                nc.sync.dma_start(out=xp[:, i], in_=x_col[b, i])
            for t in range(2):
                ps = cps.tile([128, 2, 512], F32, tag="cpsum")
                for nh in range(2):
                    nc.tensor.matmul(ps[:, nh, 0:384], ones1,
                                     cbsb[:, nh * 384:(nh + 1) * 384],
                                     start=True, stop=False)
                for i in range(27):
                    xv = xp[:, i, 128 * t:128 * t + 128]
                    for nh in range(2):
                        nc.tensor.matmul(ps[:, nh, 0:384], xv,
                                         wsb[:, i, nh * 384:(nh + 1) * 384],
                                         start=False, stop=(i == 26))
                mv, rs = _ln_stats(nc, csc, ps[:, 0, 0:384], ps[:, 1, 0:384], 128, eps)
                yt = csc.tile([128, 768], F32, tag="cyt")
                nc.vector.scalar_tensor_tensor(out=yt, in0=ps[:, :, 0:384],
                                               scalar=mv[:, 0:1], in1=pgc,
                                               op0=ALU.subtract, op1=ALU.mult)
                nc.vector.scalar_tensor_tensor(out=ym[:, 2 * b + t], in0=yt,
                                               scalar=rs, in1=gsb[:, t],
                                               op0=ALU.mult, op1=ALU.add)
        for cm in (csc_cm, cps_cm, cx_cm, cw_cm):
            cm.__exit__(None, None, None)

        # ---------------- P2: LN1 + transpose -> hT ----------------
        l1_cm = tc.tile_pool(name="l1", bufs=3)
        l1p_cm = tc.tile_pool(name="l1p", bufs=2, space="PSUM")
        l1 = l1_cm.__enter__()
        l1p = l1p_cm.__enter__()
        for ci, (t0, ts) in enumerate(CH):
            src = ym[:, ci] if ci < 16 else yx
            mv, rs = _ln_stats(nc, l1, src[:ts, 0:384], src[:ts, 384:768], ts, eps)
            hb = l1.tile([128, 768], BF16, tag="l1h")
            nc.vector.tensor_scalar(out=hb[:ts], in0=src[:ts], scalar1=mv[:ts, 0:1],
                                    scalar2=rs[:ts], op0=ALU.subtract, op1=ALU.mult)
            for k in range(6):
                tp = l1p.tile([128, 128], BF16, tag="l1t")
                nc.tensor.transpose(tp[:, :ts], hb[:ts, k * 128:(k + 1) * 128],
                                    idb[:ts, :ts])
                nc.vector.tensor_copy(hT[:, k, t0:t0 + ts], tp[:, :ts])
        l1p_cm.__exit__(None, None, None)
        l1_cm.__exit__(None, None, None)

        # ---------------- P3: QKV -> qkT (q,k) + vT (v transposed) ----------------
        pqk_cm = tc.tile_pool(name="qktp", bufs=1, side="right")
        pqk = pqk_cm.__enter__()
        qkT = pqk.tile([128, 12, 2056], BF16)
        vT = pqk.tile([128, 16, 12, 65], BF16)   # [key-chunk part, chunk, head, hd+ones]
        vtx = pqk.tile([1, 8, 12, 65], BF16)     # extra-key rows
        nc.vector.memset(vT[:, :, :, 64:65], 1.0)
        nc.vector.memset(vtx[:, :, :, 64:65], 1.0)
        qw_cm = tc.tile_pool(name="qw", bufs=1)
        qs_cm = tc.tile_pool(name="qsc", bufs=2)
        qp_cm = tc.tile_pool(name="qp", bufs=3, space="PSUM")
        vg_cm = tc.tile_pool(name="vg", bufs=2, space="PSUM")
        qw = qw_cm.__enter__()
        qsc = qs_cm.__enter__()
        qp = qp_cm.__enter__()
        vg = vg_cm.__enter__()
        wq = qw.tile([128, 6, 2304], BF16)
        for k in range(6):
            nc.sync.dma_start(out=wq[:, k], in_=qkvw[k])
        qb = qw.tile([128, 18], F32)
        nc.sync.dma_start(out=qb, in_=qkvb_t)
        vbsb = qw.tile([1, 768], BF16)
        nc.sync.dma_start(out=vbsb, in_=vb_d)
        for (c0, cs) in CC:
            for dch in range(12):
                ps = qp.tile([128, 512], F32, tag="qps")
                for k in range(6):
                    nc.tensor.matmul(ps[:, :cs], wq[:, k, dch * 128:(dch + 1) * 128],
                                     hT[:, k, c0:c0 + cs], start=(k == 0), stop=(k == 5))
                nc.vector.tensor_scalar_add(out=qkT[:, dch, c0:c0 + cs],
                                            in0=ps[:, :cs], scalar1=qb[:, dch:dch + 1])
        for ci, (t0, ts) in enumerate(CH):
            ps = vg.tile([128, 2, 512], F32, tag="vps")
            for nh in range(2):
                nc.tensor.matmul(ps[:ts, nh, 0:384], ones1[:, :ts],
                                 vbsb[:, nh * 384:(nh + 1) * 384],
                                 start=True, stop=False)
                for k in range(6):
                    nc.tensor.matmul(ps[:ts, nh, 0:384], hT[:, k, t0:t0 + ts],
                                     wq[:, k, 1536 + nh * 384:1536 + (nh + 1) * 384],
                                     start=False, stop=(k == 5))
            if ci < 16:
                nc.vector.tensor_copy(vT[:ts, ci, :, 0:64], ps[:ts, :, 0:384])
            else:
                stg = qsc.tile([8, 768], BF16, tag="vstg")
                nc.vector.tensor_copy(stg, ps[:8, :, 0:384])
                nc.sync.dma_start(out=vtx[:, :, :, 0:64], in_=stg)
        vg_cm.__exit__(None, None, None)
        qp_cm.__exit__(None, None, None)
        qs_cm.__exit__(None, None, None)
        qw_cm.__exit__(None, None, None)
        pht_cm.__exit__(None, None, None)      # hT dead

        # ---------------- P4: attention -> oT ----------------
        pot_cm = tc.tile_pool(name="otp", bufs=1)
        pot = pot_cm.__enter__()
        oT = pot.tile([128, 6, 2056], BF16)
        ab_cm = tc.tile_pool(name="ab", bufs=1)
        eb_cm = tc.tile_pool(name="ebp", bufs=2)
        as_cm = tc.tile_pool(name="asc", bufs=3)
        s2_cm = tc.tile_pool(name="s2p", bufs=2)
        ap1_cm = tc.tile_pool(name="ap1", bufs=2, space="PSUM")   # qkps: 4 banks
        ap2_cm = tc.tile_pool(name="ap2", bufs=1, space="PSUM")   # qk1+rps: 2 banks
        ap3_cm = tc.tile_pool(name="ap3", bufs=2, space="PSUM")   # avps: 2 banks
        ab = ab_cm.__enter__()
        ebp = eb_cm.__enter__()
        asc = as_cm.__enter__()
        s2p = s2_cm.__enter__()
        ap1 = ap1_cm.__enter__()
        ap2 = ap2_cm.__enter__()
        ap3 = ap3_cm.__enter__()
        eb1sb = ab.tile([1, 12, 257], BF16)
        nc.sync.dma_start(out=eb1sb, in_=bT_d[:, 256:257, :])
        e2 = ab.tile([2, 128], BF16)
        nc.sync.dma_start(out=e2, in_=e2_d)
        for hk in range(6):
            s2 = s2p.tile([2, 2056], BF16, tag="s2")
            for hp in range(2):
                h, po = 2 * hk + hp, hp * 64
                ebsb = ebp.tile([128, 2, 257], BF16, tag="ebsb")
                for ci2 in range(2):
                    nc.sync.dma_start(out=ebsb[:, ci2],
                                      in_=bT_d[h, ci2 * 128:(ci2 + 1) * 128, :])
                for b in range(BL):
                    qs = qkT[po:po + 64, hk, 256 * b:256 * b + 256]
                    qx = qkT[po:po + 64, hk, 2048 + b:2049 + b]
                    kx = qkT[po:po + 64, 6 + hk, 2048 + b:2049 + b]
                    qkps = ap1.tile([128, 2, 512], F32, tag="qkps")
                    for ci2 in range(2):
                        kst = qkT[po:po + 64, 6 + hk,
                                  256 * b + 128 * ci2:256 * b + 128 * ci2 + 128]
                        nc.tensor.matmul(qkps[:, ci2, 0:256], kst, qs,
                                         start=True, stop=False, skip_group_check=True)
                        nc.tensor.matmul(qkps[:, ci2, 256:257], kst, qx,
                                         start=False, stop=True, skip_group_check=True)
                    qk1 = ap2.tile([1, 512], F32, tag="qk1")
                    nc.tensor.matmul(qk1[:, 0:256], kx, qs,
                                     start=True, stop=False, skip_group_check=True)
                    nc.tensor.matmul(qk1[:, 256:257], kx, qx,
                                     start=False, stop=True, skip_group_check=True)
                    pe = asc.tile([128, 2, 257], BF16, tag="pe")
                    nc.scalar.activation(out=pe, in_=qkps[:, :, 0:257], func=AF.Exp)
                    pe1 = asc.tile([1, 257], BF16, tag="pe1")
                    nc.scalar.activation(out=pe1, in_=qk1[:, 0:257], func=AF.Exp)
                    nc.gpsimd.tensor_mul(pe, pe, ebsb)
                    nc.gpsimd.tensor_mul(pe1, pe1, eb1sb[:, h])
                    avps = ap3.tile([65, 512], F32, tag="avps")
                    nc.tensor.matmul(avps[:, 0:257], vT[:, 2 * b, h], pe[:, 0],
                                     start=True, stop=False)
                    nc.tensor.matmul(avps[:, 0:257], vT[:, 2 * b + 1, h], pe[:, 1],
                                     start=False, stop=False)
                    nc.tensor.matmul(avps[:, 0:257], vtx[:, b, h], pe1,
                                     start=False, stop=True)
                    ou = asc.tile([65, 257], BF16, tag="ou")
                    nc.vector.tensor_copy(ou, avps[:, 0:257])
                    nc.sync.dma_start(out=oT[po:po + 64, hk, 256 * b:256 * b + 256],
                                      in_=ou[0:64, 0:256])
                    nc.sync.dma_start(out=oT[po:po + 64, hk, 2048 + b:2049 + b],
                                      in_=ou[0:64, 256:257])
                    nc.sync.dma_start(out=s2[hp:hp + 1, 256 * b:256 * b + 256],
                                      in_=ou[64:65, 0:256])
                    nc.sync.dma_start(out=s2[hp:hp + 1, 2048 + b:2049 + b],
                                      in_=ou[64:65, 256:257])
            with nc.allow_low_precision(reason="softmax denom recip bf16; tol 2e-2"):
                nc.vector.reciprocal(s2, s2)
            for (c0, cs) in CC:
                rp = ap2.tile([128, 512], F32, tag="rps")
                nc.tensor.matmul(rp[:, :cs], e2, s2[:, c0:c0 + cs],
                                 start=True, stop=True)
                nc.vector.tensor_mul(oT[:, hk, c0:c0 + cs], oT[:, hk, c0:c0 + cs],
                                     rp[:, :cs])
        for cm in (ap3_cm, ap2_cm, ap1_cm, s2_cm, as_cm, eb_cm, ab_cm):
            cm.__exit__(None, None, None)
        pqk_cm.__exit__(None, None, None)      # qkT dead

        # ---------------- P5: proj + residual + LN2 + T -> ym, h2T ----------------
        ph2_cm = tc.tile_pool(name="h2tp", bufs=1, side="right")
        ph2 = ph2_cm.__enter__()
        h2T = ph2.tile([128, 6, 2056], BF16)
        pw_cm = tc.tile_pool(name="pw", bufs=1)
        pa_cm = tc.tile_pool(name="pa", bufs=3)
        pp_cm = tc.tile_pool(name="pp", bufs=2, space="PSUM")
        pw = pw_cm.__enter__()
        pa = pa_cm.__enter__()
        pp = pp_cm.__enter__()
        wp = pw.tile([128, 6, 768], BF16)
        for k in range(6):
            nc.sync.dma_start(out=wp[:, k], in_=projw[k])
        pbsb = pw.tile([1, 768], BF16)
        nc.sync.dma_start(out=pbsb, in_=projb)
        for ci, (t0, ts) in enumerate(CH):
            ydst = ym[:, ci] if ci < 16 else yx
            ps = pp.tile([128, 2, 512], F32, tag="pps")
            for nh in range(2):
                nc.tensor.matmul(ps[:ts, nh, 0:384], ones1[:, :ts],
                                 pbsb[:, nh * 384:(nh + 1) * 384],
                                 start=True, stop=False)
                for k in range(6):
                    nc.tensor.matmul(ps[:ts, nh, 0:384], oT[:, k, t0:t0 + ts],
                                     wp[:, k, nh * 384:(nh + 1) * 384],
                                     start=False, stop=(k == 5))
            nc.vector.tensor_add(ydst[:ts], ydst[:ts], ps[:ts, :, 0:384])
            mv, rs = _ln_stats(nc, pa, ydst[:ts, 0:384], ydst[:ts, 384:768], ts, eps)
            hb = pa.tile([128, 768], BF16, tag="p5h")
            nc.vector.tensor_scalar(out=hb[:ts], in0=ydst[:ts], scalar1=mv[:ts, 0:1],
                                    scalar2=rs[:ts], op0=ALU.subtract, op1=ALU.mult)
            for k in range(6):
                tp = pp.tile([128, 128], BF16, tag="p5t")
                nc.tensor.transpose(tp[:, :ts], hb[:ts, k * 128:(k + 1) * 128],
                                    idb[:ts, :ts])
                nc.vector.tensor_copy(h2T[:, k, t0:t0 + ts], tp[:, :ts])
        pp_cm.__exit__(None, None, None)
        pa_cm.__exit__(None, None, None)
        pw_cm.__exit__(None, None, None)
        pot_cm.__exit__(None, None, None)      # oT dead

        # ---------------- P6/P7: MLP fused per column chunk ----------------
        fw_cm = tc.tile_pool(name="fw", bufs=1)
        fa_cm = tc.tile_pool(name="fa", bufs=1)
        fo_cm = tc.tile_pool(name="fo", bufs=2)
        f1p_cm = tc.tile_pool(name="f1p", bufs=2, space="PSUM")
        f2p_cm = tc.tile_pool(name="f2p", bufs=2, space="PSUM")
        fw = fw_cm.__enter__()
        fa = fa_cm.__enter__()
        fo = fo_cm.__enter__()
        f1p = f1p_cm.__enter__()
        f2p = f2p_cm.__enter__()
        w1 = fw.tile([128, 6, MLP], BF16)
        for k in range(6):
            nc.sync.dma_start(out=w1[:, k], in_=fc1w[k])
        b1t = fw.tile([128, 24], F32)
        nc.sync.dma_start(out=b1t, in_=fc1b_t)
        w2 = fw.tile([128, 24, 768], BF16)
        for k in range(24):
            nc.sync.dma_start(out=w2[:, k], in_=fc2w[k])
        b2sb = fw.tile([1, 768], BF16)
        nc.sync.dma_start(out=b2sb, in_=fc2b)
        for (c0, cs) in CC:
            h3 = fa.tile([128, 24, 512], BF16, tag="h3")
            for dch in range(24):
                ps = f1p.tile([128, 512], F32, tag="f1ps")
                for k in range(6):
                    nc.tensor.matmul(ps[:, :cs], w1[:, k, dch * 128:(dch + 1) * 128],
                                     h2T[:, k, c0:c0 + cs], start=(k == 0), stop=(k == 5))
                nc.scalar.activation(out=h3[:, dch, :cs], in_=ps[:, :cs],
                                     func=AF.Gelu, bias=b1t[:, dch:dch + 1])
            for s in range(4 if cs == 512 else 1):
                ts = min(128, cs - 128 * s)
                ci = (c0 + 128 * s) // 128
                ysrc = ym[:, ci] if ci < 16 else yx
                ps2 = f2p.tile([128, 2, 512], F32, tag="f2ps")
                for nh in range(2):
                    nc.tensor.matmul(ps2[:ts, nh, 0:384], ones1[:, :ts],
                                     b2sb[:, nh * 384:(nh + 1) * 384],
                                     start=True, stop=False)
                    for k in range(24):
                        nc.tensor.matmul(ps2[:ts, nh, 0:384],
                                         h3[:, k, 128 * s:128 * s + ts],
                                         w2[:, k, nh * 384:(nh + 1) * 384],
                                         start=False, stop=(k == 23))
                ot = fo.tile([128, 768], F32, tag="fout")
                nc.vector.tensor_add(ot[:ts], ysrc[:ts], ps2[:ts, :, 0:384])
                nc.sync.dma_start(out=out_d[c0 + 128 * s:c0 + 128 * s + ts, :],
                                  in_=ot[:ts])
        for cm in (f2p_cm, f1p_cm, fo_cm, fa_cm, fw_cm):
            cm.__exit__(None, None, None)
        ph2_cm.__exit__(None, None, None)
        pr_cm.__exit__(None, None, None)
        pc_cm.__exit__(None, None, None)

    nc.compile()
    return nc


def kernel(x, H, W, geo_bias, extra_token, conv_w, conv_b, pe_g, pe_b,
           n1_g, n1_b, qkv_w, rpb_table, proj_w, proj_b, n2_g, n2_b,
           fc1_w, fc1_b, fc2_w, fc2_b):
    x = np.asarray(x, np.float32)
    f = lambda a: np.asarray(a, np.float32)
    geo_bias, extra_token = f(geo_bias), f(extra_token)
    conv_w, conv_b, pe_g, pe_b = f(conv_w), f(conv_b), f(pe_g), f(pe_b)
    n1_g, n1_b, qkv_w, rpb_table = f(n1_g), f(n1_b), f(qkv_w), f(rpb_table)
    proj_w, proj_b, n2_g, n2_b = f(proj_w), f(proj_b), f(n2_g), f(n2_b)
    fc1_w, fc1_b, fc2_w, fc2_b = f(fc1_w), f(fc1_b), f(fc2_w), f(fc2_b)

    if "nc" not in _CACHE:
        _CACHE["nc"] = build()
    nc = _CACHE["nc"]

    # host-side weight prep (layout only; LN gain/bias folds are exact)
    cw = conv_w.transpose(2, 3, 1, 0).reshape(3, 3, 3, 128, 768).reshape(27, 128, 768)
    qkv_wf = qkv_w * n1_g[None, :]
    qkv_wf[:D] *= HD ** -0.5
    qkv_b = qkv_w @ n1_b
    qkv_b[:D] *= HD ** -0.5
    fc1_wf = fc1_w * n2_g[None, :]
    fc1_bf = fc1_b + fc1_w @ n2_b
    bias_full = _rel_bias(rpb_table)                     # [12, q, k]
    perm = np.r_[1:257, 0]
    bp = np.exp(bias_full[:, perm][:, :, perm])          # exp(bias) [12, q', k']
    bT = np.ascontiguousarray(bp.transpose(0, 2, 1))     # [12, k', q']

    common = {
        "convw": cw.astype(BF),
        "convb": conv_b[None, :].astype(BF),
        "peg_bc": np.tile(pe_g[None, :], (128, 1)).astype(np.float32),
        "geo2": (geo_bias[0, 1:, :] + pe_b[None, :]).reshape(2, 128, 768).astype(np.float32),
        "y0row": (extra_token[0] + geo_bias[0, :1, :]).astype(np.float32),
        "qkvw": qkv_wf.T.reshape(6, 128, 2304).astype(BF),
        "qkvb_t": np.ascontiguousarray(qkv_b.reshape(18, 128).T).astype(np.float32),
        "vb": qkv_b[None, 1536:].astype(BF),
        "e2": np.repeat(np.eye(2, dtype=np.float32), 64, axis=1).astype(BF),
        "bT_d": bT.astype(BF),
        "projw": proj_w.T.reshape(6, 128, 768).astype(BF),
        "projb": proj_b[None, :].astype(BF),
        "fc1w": fc1_wf.T.reshape(6, 128, MLP).astype(BF),
        "fc1b_t": np.ascontiguousarray(fc1_bf.reshape(24, 128).T).astype(np.float32),
        "fc2w": fc2_w.T.reshape(24, 128, 768).astype(BF),
        "fc2b": fc2_b[None, :].astype(BF),
    }
    xr = x.reshape(B, 3, 128, 32, 32)
    xpad = np.zeros((B, 3, 128, 34, 34), np.float32)
    xpad[:, :, :, 1:33, 1:33] = xr
    xcol = np.empty((B, 9, 3, 128, 256), np.float32)
    for kh in range(3):
        for kw in range(3):
            xcol[:, kh * 3 + kw] = xpad[:, :, :, kh:kh + 32:2, kw:kw + 32:2].reshape(
                B, 3, 128, 256)
    xcol = xcol.reshape(B, 27, 128, 256).astype(BF)
    in_maps = []
    for c in range(8):
        in_maps.append({"x_col": xcol[c * BL:(c + 1) * BL], **common})

    global _LAST_MAPS
    _LAST_MAPS = in_maps
    res = bass_utils.run_bass_kernel_spmd(nc, in_maps, core_ids=list(range(8)))
    # un-permute: kernel row j: main j = b*256 + (n-1); extra j = 2048 + b
    idx = np.empty(NT, np.int64)
    for b in range(BL):
        idx[b * N] = NMAIN + b
        idx[b * N + 1:(b + 1) * N] = np.arange(b * 256, b * 256 + 256)
    out = np.concatenate([r["out_d"][idx].reshape(BL, N, D) for r in res.results], axis=0)
    return out.astype(np.float32)
